# revision 8
# baseline (speedup 1.0000x reference)
"""WaveNet-style gated dilated conv layer on 8 Trainium2 NeuronCores.

Strategy: data-parallel over batch (B=8 -> 1 batch element per core).
Per core (batch b):
  z_tanh = sum_k Wc_tanh[k] @ x[:, t-d*(2-k)] + Wcond_tanh @ cond + bias
  z_sig  = likewise for the second half of the 2R conv channels
  h      = tanh(z_tanh) * sigmoid(z_sig)
  out    = W_out @ h, skip = W_skip @ h  (1x1 convs)
All matmuls run in bf16 with fp32 PSUM accumulation.  x and cond are cast
to bf16 on host to halve HBM->SBUF traffic; x is also causal-padded on
host so no on-chip memset is needed.  z biases fold into the activation
instruction; out/skip biases (zero-filled per spec) are added on host
only if nonzero.

TRN2 matmul instructions only have room for a single semaphore wait, so
the kernel is structured so no matmul ever needs two: input DMAs are
"observed" by the PE via standalone ldweights instructions before the
first matmul that would otherwise combine a DMA wait with a PSUM WAR
wait.
"""

import sys

for _p in ("/opt/trn_rl_repo",):
    if _p not in sys.path:
        sys.path.append(_p)

from contextlib import ExitStack

import ml_dtypes
import numpy as np

import concourse.bacc as bacc
import concourse.bass as bass
import concourse.tile as tile
from concourse import mybir
from concourse.bass_utils import run_bass_kernel_spmd

B, CIN, T = 8, 128, 16384
R, S, CC, KW = 128, 128, 80, 3
NT = 512           # time-tile width (one PSUM bank of fp32)
NTILES = T // NT
NCHUNKS = 8        # DMA chunks for the big input loads
N_CORES = 8

BF16 = mybir.dt.bfloat16
FP32 = mybir.dt.float32
AF = mybir.ActivationFunctionType

_built = {}
_TRACE = False        # set True (e.g. by a test harness) to capture an NTFF profile
_last_results = None  # BassKernelResults of the most recent run


def _build(dilation: int) -> bass.Bass:
    pad = dilation * (KW - 1)
    xw = pad + T                    # padded x width
    xch = -(-xw // NCHUNKS)         # x chunk width (last chunk may be short)
    cch = -(-T // NCHUNKS)          # cond chunk width

    nc = bacc.Bacc("TRN2", target_bir_lowering=False, debug=False, num_devices=N_CORES)

    x = nc.declare_dram_parameter("x", [CIN, xw], BF16, isOutput=False)
    cond = nc.declare_dram_parameter("cond", [CC, T], BF16, isOutput=False)
    # packed lhsT weights (already transposed to [Cin, Cout] on host)
    wconv = nc.declare_dram_parameter("wconv", [CIN, 2 * KW * R], BF16, isOutput=False)
    wcond = nc.declare_dram_parameter("wcond", [CC, 2 * R], BF16, isOutput=False)
    wos = nc.declare_dram_parameter("wos", [R, R + S], BF16, isOutput=False)
    zbias = nc.declare_dram_parameter("zbias", [R, 2], FP32, isOutput=False)

    out = nc.declare_dram_parameter("out", [R, T], FP32, isOutput=True)
    skip = nc.declare_dram_parameter("skip", [S, T], FP32, isOutput=True)

    with tile.TileContext(nc) as tc, ExitStack() as ctx:
        consts = ctx.enter_context(tc.tile_pool(name="consts", bufs=1))
        bigbuf = ctx.enter_context(tc.tile_pool(name="bigbuf", bufs=1))
        hpool = ctx.enter_context(tc.tile_pool(name="h", bufs=3))
        zpsum = ctx.enter_context(tc.tile_pool(name="zpsum", bufs=2, space="PSUM"))
        opsum = ctx.enter_context(tc.tile_pool(name="opsum", bufs=2, space="PSUM"))

        wconv_sb = consts.tile([CIN, 2 * KW * R], BF16)
        nc.sync.dma_start(wconv_sb[:], wconv[:])
        wcond_sb = consts.tile([CC, 2 * R], BF16)
        nc.sync.dma_start(wcond_sb[:], wcond[:])
        wos_sb = consts.tile([R, R + S], BF16)
        nc.sync.dma_start(wos_sb[:], wos[:])
        zbias_sb = consts.tile([R, 2], FP32)
        nc.sync.dma_start(zbias_sb[:], zbias[:])

        xp = bigbuf.tile([CIN, xw], BF16, tag="xp")
        for c in range(NCHUNKS):
            a, b = c * xch, min((c + 1) * xch, xw)
            nc.sync.dma_start(xp[:, a:b], x[:, a:b])
        cd = bigbuf.tile([CC, T], BF16, tag="cd")
        for c in range(NCHUNKS):
            a, b = c * cch, min((c + 1) * cch, T)
            nc.sync.dma_start(cd[:, a:b], cond[:, a:b])

        # PE-side DMA-semaphore observation (see module docstring):
        # before the tiles that read chunk c, issue a bare ldweights that
        # reads 128 columns of it, carrying the (sole) DMA wait.
        x_seen = [False] * NCHUNKS
        c_seen = [False] * NCHUNKS

        def observe(i):
            lo, hi = i * NT, i * NT + pad + NT  # padded x cols tile i reads
            for c in range(lo // xch, min((hi - 1) // xch, NCHUNKS - 1) + 1):
                if not x_seen[c]:
                    x_seen[c] = True
                    nc.tensor.ldweights(xp[:, c * xch : c * xch + 128])
            for c in ((i * NT) // cch,):
                if not c_seen[c]:
                    c_seen[c] = True
                    nc.tensor.ldweights(cd[:, c * cch : c * cch + 128])

        for i in range(NTILES):
            t0 = i * NT
            observe(i)
            ztan = zpsum.tile([R, NT], FP32, tag="ztan")
            zsig = zpsum.tile([R, NT], FP32, tag="zsig")
            for k in range(KW):
                xs = xp[:, t0 + dilation * k : t0 + dilation * k + NT]
                nc.tensor.matmul(
                    ztan[:], wconv_sb[:, k * R : (k + 1) * R], xs,
                    start=(k == 0), stop=False,
                )
            nc.tensor.matmul(
                ztan[:], wcond_sb[:, 0:R], cd[:, t0 : t0 + NT],
                start=False, stop=True,
            )
            for k in range(KW):
                xs = xp[:, t0 + dilation * k : t0 + dilation * k + NT]
                nc.tensor.matmul(
                    zsig[:], wconv_sb[:, (KW + k) * R : (KW + k + 1) * R], xs,
                    start=(k == 0), stop=False,
                )
            nc.tensor.matmul(
                zsig[:], wcond_sb[:, R : 2 * R], cd[:, t0 : t0 + NT],
                start=False, stop=True,
            )

            th = hpool.tile([R, NT], BF16, tag="th")
            nc.scalar.activation(th[:], ztan[:], AF.Tanh, bias=zbias_sb[:, 0:1])
            sg = hpool.tile([R, NT], BF16, tag="sg")
            nc.scalar.activation(sg[:], zsig[:], AF.Sigmoid, bias=zbias_sb[:, 1:2])
            h = hpool.tile([R, NT], BF16, tag="h")
            nc.vector.tensor_mul(h[:], th[:], sg[:])

            po = opsum.tile([R, NT], FP32, tag="po")
            nc.tensor.matmul(po[:], wos_sb[:, 0:R], h[:], start=True, stop=True)
            ps = opsum.tile([S, NT], FP32, tag="ps")
            nc.tensor.matmul(ps[:], wos_sb[:, R : R + S], h[:], start=True, stop=True)

            osb = hpool.tile([R, NT], FP32, tag="osb")
            nc.vector.tensor_copy(osb[:], po[:])
            ssb = hpool.tile([S, NT], FP32, tag="ssb")
            nc.vector.tensor_copy(ssb[:], ps[:])
            nc.sync.dma_start(out[:, t0 : t0 + NT], osb[:])
            nc.sync.dma_start(skip[:, t0 : t0 + NT], ssb[:])

    nc.compile()
    return nc


def _pack_weights(w_conv, w_cond, w_out, w_skip, b_conv, b_cond):
    bf = ml_dtypes.bfloat16
    wconv_p = np.empty((CIN, 2 * KW * R), dtype=bf)
    for k in range(KW):
        wconv_p[:, k * R : (k + 1) * R] = w_conv[0:R, :, k].T.astype(bf)
        wconv_p[:, (KW + k) * R : (KW + k + 1) * R] = w_conv[R : 2 * R, :, k].T.astype(bf)
    wcond_p = np.concatenate(
        [w_cond[0:R, :, 0].T, w_cond[R : 2 * R, :, 0].T], axis=1
    ).astype(bf)
    wos_p = np.concatenate([w_out[:, :, 0].T, w_skip[:, :, 0].T], axis=1).astype(bf)
    zbias_p = np.stack(
        [b_conv[:R] + b_cond[:R], b_conv[R:] + b_cond[R:]], axis=1
    ).astype(np.float32)
    return wconv_p, wcond_p, wos_p, zbias_p


def kernel(**inputs):
    x = np.asarray(inputs["x"], dtype=np.float32)
    cond = np.asarray(inputs["cond"], dtype=np.float32)
    w_conv = np.asarray(inputs["w_conv"], dtype=np.float32)
    b_conv = np.asarray(inputs["b_conv"], dtype=np.float32)
    w_cond = np.asarray(inputs["w_cond"], dtype=np.float32)
    b_cond = np.asarray(inputs["b_cond"], dtype=np.float32)
    w_out = np.asarray(inputs["w_out"], dtype=np.float32)
    b_out = np.asarray(inputs["b_out"], dtype=np.float32)
    w_skip = np.asarray(inputs["w_skip"], dtype=np.float32)
    b_skip = np.asarray(inputs["b_skip"], dtype=np.float32)
    dilation = int(np.asarray(inputs["dilation"]))
    pad = dilation * (KW - 1)

    if dilation not in _built:
        _built[dilation] = _build(dilation)
    nc = _built[dilation]

    wconv_p, wcond_p, wos_p, zbias_p = _pack_weights(
        w_conv, w_cond, w_out, w_skip, b_conv, b_cond
    )
    bf = ml_dtypes.bfloat16
    xb = np.zeros((B, CIN, pad + T), dtype=bf)
    xb[:, :, pad:] = x.astype(bf)
    cb = np.ascontiguousarray(cond.astype(bf))

    in_maps = [
        {
            "x": xb[b],
            "cond": cb[b],
            "wconv": wconv_p,
            "wcond": wcond_p,
            "wos": wos_p,
            "zbias": zbias_p,
        }
        for b in range(B)
    ]
    br = run_bass_kernel_spmd(nc, in_maps, list(range(N_CORES)), trace=_TRACE)
    global _last_results
    _last_results = br
    res = br.results
    output = np.stack([res[b]["out"] for b in range(B)])
    skip = np.stack([res[b]["skip"] for b in range(B)])
    if b_out.any():
        output = output + b_out[None, :, None]
    if b_skip.any():
        skip = skip + b_skip[None, :, None]
    return (output, skip)


# revision 11
# speedup vs baseline: 1.0509x; 1.0509x over previous
"""WaveNet-style gated dilated conv layer on 8 Trainium2 NeuronCores.

Strategy: data-parallel over batch (B=8 -> 1 batch element per core).
Per core (batch b):
  z_tanh = sum_k Wc_tanh[k] @ x[:, t-d*(2-k)] + Wcond_tanh @ cond + bias
  z_sig  = likewise for the second half of the 2R conv channels
  h      = tanh(z_tanh) * sigmoid(z_sig)
  out    = W_out @ h, skip = W_skip @ h  (1x1 convs)
All matmuls run in bf16 with fp32 PSUM accumulation.  x and cond are cast
to bf16 on host to halve HBM->SBUF traffic; x is also causal-padded on
host so no on-chip memset is needed.  z biases fold into the activation
instruction; out/skip biases (zero-filled per spec) are added on host
only if nonzero.

TRN2 matmul instructions only have room for a single semaphore wait, so
the kernel is structured so no matmul ever needs two: input DMAs are
"observed" by the PE via standalone ldweights instructions before the
first matmul that would otherwise combine a DMA wait with a PSUM WAR
wait.
"""

import sys

for _p in ("/opt/trn_rl_repo",):
    if _p not in sys.path:
        sys.path.append(_p)

from contextlib import ExitStack

import ml_dtypes
import numpy as np

import concourse.bacc as bacc
import concourse.bass as bass
import concourse.tile as tile
from concourse import mybir
from concourse.bass_utils import run_bass_kernel_spmd

B, CIN, T = 8, 128, 16384
R, S, CC, KW = 128, 128, 80, 3
NT = 512           # time-tile width (one PSUM bank of fp32)
NTILES = T // NT
NCHUNKS = 8        # DMA chunks for the big input loads
N_CORES = 8

BF16 = mybir.dt.bfloat16
FP32 = mybir.dt.float32
AF = mybir.ActivationFunctionType

_built = {}
_TRACE = False        # set True (e.g. by a test harness) to capture an NTFF profile
_last_results = None  # BassKernelResults of the most recent run


CW = 1024            # streaming chunk width (x cols per chunk)
GT = CW // NT        # tiles per chunk group
NCH = T // CW        # number of chunks
PREFETCH = 2         # chunk lookahead beyond the current group


def _build(dilation: int) -> bass.Bass:
    pad = dilation * (KW - 1)

    nc = bacc.Bacc("TRN2", target_bir_lowering=False, debug=False, num_devices=N_CORES)

    x = nc.declare_dram_parameter("x", [CIN, pad + T], BF16, isOutput=False)
    cond = nc.declare_dram_parameter("cond", [CC, T], BF16, isOutput=False)
    # packed lhsT weights (already transposed to [Cin, Cout] on host)
    wconv = nc.declare_dram_parameter("wconv", [CIN, 2 * KW * R], BF16, isOutput=False)
    wcond = nc.declare_dram_parameter("wcond", [CC, 2 * R], BF16, isOutput=False)
    wos = nc.declare_dram_parameter("wos", [R, R + S], BF16, isOutput=False)
    zbias = nc.declare_dram_parameter("zbias", [R, 2], FP32, isOutput=False)

    out = nc.declare_dram_parameter("out", [R, T], FP32, isOutput=True)
    skip = nc.declare_dram_parameter("skip", [S, T], FP32, isOutput=True)

    with tile.TileContext(nc) as tc, ExitStack() as ctx:
        consts = ctx.enter_context(tc.tile_pool(name="consts", bufs=1))
        inpool = ctx.enter_context(tc.tile_pool(name="inp", bufs=PREFETCH + 2))
        hpool = ctx.enter_context(tc.tile_pool(name="h", bufs=3))
        opool = ctx.enter_context(tc.tile_pool(name="o", bufs=2))
        zpsum = ctx.enter_context(tc.tile_pool(name="zpsum", bufs=2, space="PSUM"))
        opsum = ctx.enter_context(tc.tile_pool(name="opsum", bufs=2, space="PSUM"))

        wconv_sb = consts.tile([CIN, 2 * KW * R], BF16)
        nc.sync.dma_start(wconv_sb[:], wconv[:])
        wcond_sb = consts.tile([CC, 2 * R], BF16)
        nc.sync.dma_start(wcond_sb[:], wcond[:])
        wos_sb = consts.tile([R, R + S], BF16)
        nc.sync.dma_start(wos_sb[:], wos[:])
        zbias_sb = consts.tile([R, 2], FP32)
        nc.sync.dma_start(zbias_sb[:], zbias[:])

        # Warm-up during the input-load head: ~9 matmuls on uninitialized
        # SBUF kick the PE HAM to 8/8 before real work arrives, and two
        # 1-column activations trigger the tanh/sigmoid table load (~2.7us).
        garbage = consts.tile([CIN, NT], BF16)
        act_sink = consts.tile([R, 1], FP32)
        nc.vector.memset(garbage[:], 0.0)
        nc.vector.memset(act_sink[:], 0.0)
        for _ in range(9):
            wz = zpsum.tile([R, NT], FP32, tag="ztan")
            nc.tensor.matmul(wz[:], garbage[:, 0:R], garbage[:], start=True, stop=True)
        nc.scalar.activation(act_sink[:], act_sink[:], AF.Tanh, bias=zbias_sb[:, 0:1])
        nc.scalar.activation(act_sink[:], act_sink[:], AF.Sigmoid, bias=zbias_sb[:, 1:2])

        # Streaming input chunks: chunk g covers x cols [g*CW, (g+1)*CW) plus
        # a causal halo of `pad` cols on the left (re-read from DRAM).  The
        # limited pool slots (WAR deps) pace the DMAs against PE progress.
        xc_tiles = [None] * NCH
        cc_tiles = [None] * NCH

        def load_chunk(g):
            xc = inpool.tile([CIN, pad + CW], BF16, tag="xc")
            nc.sync.dma_start(xc[:], x[:, g * CW : g * CW + pad + CW])
            cc = inpool.tile([CC, CW], BF16, tag="cc")
            nc.sync.dma_start(cc[:], cond[:, g * CW : (g + 1) * CW])
            xc_tiles[g], cc_tiles[g] = xc, cc

        for g in range(min(PREFETCH + 1, NCH)):
            load_chunk(g)

        for i in range(NTILES):
            g, l0 = i // GT, (i % GT) * NT
            if i % GT == 0:
                if g + PREFETCH + 1 < NCH and xc_tiles[g + PREFETCH + 1] is None:
                    load_chunk(g + PREFETCH + 1)
                xc, cc = xc_tiles[g], cc_tiles[g]
                # let PE observe the chunk DMA sems on a standalone ldweights
                # so no accumulating matmul needs two waits
                nc.tensor.ldweights(xc[:, 0:R])
                nc.tensor.ldweights(cc[:, 0:R])
            xc, cc = xc_tiles[g], cc_tiles[g]

            ztan = zpsum.tile([R, NT], FP32, tag="ztan")
            zsig = zpsum.tile([R, NT], FP32, tag="zsig")
            for k in range(KW):
                xs = xc[:, l0 + dilation * k : l0 + dilation * k + NT]
                nc.tensor.matmul(
                    ztan[:], wconv_sb[:, k * R : (k + 1) * R], xs,
                    start=(k == 0), stop=False,
                )
            nc.tensor.matmul(
                ztan[:], wcond_sb[:, 0:R], cc[:, l0 : l0 + NT],
                start=False, stop=True,
            )
            for k in range(KW):
                xs = xc[:, l0 + dilation * k : l0 + dilation * k + NT]
                nc.tensor.matmul(
                    zsig[:], wconv_sb[:, (KW + k) * R : (KW + k + 1) * R], xs,
                    start=(k == 0), stop=False,
                )
            nc.tensor.matmul(
                zsig[:], wcond_sb[:, R : 2 * R], cc[:, l0 : l0 + NT],
                start=False, stop=True,
            )

            th = hpool.tile([R, NT], BF16, tag="th")
            nc.scalar.activation(th[:], ztan[:], AF.Tanh, bias=zbias_sb[:, 0:1])
            sg = hpool.tile([R, NT], BF16, tag="sg")
            nc.scalar.activation(sg[:], zsig[:], AF.Sigmoid, bias=zbias_sb[:, 1:2])
            h = hpool.tile([R, NT], BF16, tag="h")
            nc.vector.tensor_mul(h[:], th[:], sg[:])

            po = opsum.tile([R, NT], FP32, tag="po")
            nc.tensor.matmul(po[:], wos_sb[:, 0:R], h[:], start=True, stop=True)
            ps = opsum.tile([S, NT], FP32, tag="ps")
            nc.tensor.matmul(ps[:], wos_sb[:, R : R + S], h[:], start=True, stop=True)

            # stage into a per-chunk-group output buffer; one big DMA per group
            if i % GT == 0:
                osb = opool.tile([R, CW], FP32, tag="osb")
                ssb = opool.tile([S, CW], FP32, tag="ssb")
            nc.vector.tensor_copy(osb[:, l0 : l0 + NT], po[:])
            nc.vector.tensor_copy(ssb[:, l0 : l0 + NT], ps[:])
            if i % GT == GT - 1:
                nc.sync.dma_start(out[:, g * CW : (g + 1) * CW], osb[:])
                nc.sync.dma_start(skip[:, g * CW : (g + 1) * CW], ssb[:])

    nc.compile()
    return nc


def _pack_weights(w_conv, w_cond, w_out, w_skip, b_conv, b_cond):
    bf = ml_dtypes.bfloat16
    wconv_p = np.empty((CIN, 2 * KW * R), dtype=bf)
    for k in range(KW):
        wconv_p[:, k * R : (k + 1) * R] = w_conv[0:R, :, k].T.astype(bf)
        wconv_p[:, (KW + k) * R : (KW + k + 1) * R] = w_conv[R : 2 * R, :, k].T.astype(bf)
    wcond_p = np.concatenate(
        [w_cond[0:R, :, 0].T, w_cond[R : 2 * R, :, 0].T], axis=1
    ).astype(bf)
    wos_p = np.concatenate([w_out[:, :, 0].T, w_skip[:, :, 0].T], axis=1).astype(bf)
    zbias_p = np.stack(
        [b_conv[:R] + b_cond[:R], b_conv[R:] + b_cond[R:]], axis=1
    ).astype(np.float32)
    return wconv_p, wcond_p, wos_p, zbias_p


def kernel(**inputs):
    x = np.asarray(inputs["x"], dtype=np.float32)
    cond = np.asarray(inputs["cond"], dtype=np.float32)
    w_conv = np.asarray(inputs["w_conv"], dtype=np.float32)
    b_conv = np.asarray(inputs["b_conv"], dtype=np.float32)
    w_cond = np.asarray(inputs["w_cond"], dtype=np.float32)
    b_cond = np.asarray(inputs["b_cond"], dtype=np.float32)
    w_out = np.asarray(inputs["w_out"], dtype=np.float32)
    b_out = np.asarray(inputs["b_out"], dtype=np.float32)
    w_skip = np.asarray(inputs["w_skip"], dtype=np.float32)
    b_skip = np.asarray(inputs["b_skip"], dtype=np.float32)
    dilation = int(np.asarray(inputs["dilation"]))
    pad = dilation * (KW - 1)

    if dilation not in _built:
        _built[dilation] = _build(dilation)
    nc = _built[dilation]

    wconv_p, wcond_p, wos_p, zbias_p = _pack_weights(
        w_conv, w_cond, w_out, w_skip, b_conv, b_cond
    )
    bf = ml_dtypes.bfloat16
    xb = np.zeros((B, CIN, pad + T), dtype=bf)
    xb[:, :, pad:] = x.astype(bf)
    cb = np.ascontiguousarray(cond.astype(bf))

    in_maps = [
        {
            "x": xb[b],
            "cond": cb[b],
            "wconv": wconv_p,
            "wcond": wcond_p,
            "wos": wos_p,
            "zbias": zbias_p,
        }
        for b in range(B)
    ]
    br = run_bass_kernel_spmd(nc, in_maps, list(range(N_CORES)), trace=_TRACE)
    global _last_results
    _last_results = br
    res = br.results
    output = np.stack([res[b]["out"] for b in range(B)])
    skip = np.stack([res[b]["skip"] for b in range(B)])
    if b_out.any():
        output = output + b_out[None, :, None]
    if b_skip.any():
        skip = skip + b_skip[None, :, None]
    return (output, skip)
